# revision 15
# baseline (speedup 1.0000x reference)
"""Expert-parallel MoE (soft routing) kernel for 8 TRN2 NeuronCores.

Problem (nn_EnhancedMixtureOfExperts): every expert processes the full batch,
outputs mixed by soft cluster probabilities.

    h1 = relu(x @ W1[e] + b1[e])      x:[B,D]  W1[e]:[D,H]
    h2 = relu(h1 @ W2[e] + b2[e])     W2[e]:[H,H2]
    y  = sigmoid(h2 @ W3[e] + b3[e])  W3[e]:[H2,1]
    out[b] = sum_e y[e,b] * probs[b,e]

Sharding: expert-parallel — core e computes expert e over the full batch.
x is transposed on the host to xT [D,B] so on-chip activations are stored
feature-on-partition ([feat,128-block] x [batch]) and every GEMM consumes
weights in their natural [in,out] layout as the stationary operand.
The weighted combine is 8*B MACs, done on the host after gather.

All matmuls run as float32r (full fp32 data; 1 cycle/row on the PE for
moving dim >= 256, i.e. bf16-rate fp32).
"""

import numpy as np

import concourse.bass as bass
import concourse.bacc as bacc
import concourse.mybir as mybir
from concourse.bass_utils import run_bass_kernel_spmd
from concourse.tile import TileContext

E = 8
B = 16384
D = 1024
H = 2048
H2 = 1024
NB = 512  # batch columns per chunk (one PSUM bank of fp32)

F32 = mybir.dt.float32
F32R = mybir.dt.float32r
AF = mybir.ActivationFunctionType

DBLK = D // 128   # 8
HBLK = H // 128   # 16
KBLK = H2 // 128  # 8


def build_moe_nc(batch: int = B) -> bass.Bass:
    nchunk = batch // NB
    nc = bacc.Bacc("TRN2")

    xT = nc.declare_dram_parameter("xT", [D, batch], F32R, isOutput=False)
    w1 = nc.declare_dram_parameter("w1", [D, H], F32R, isOutput=False)
    w2 = nc.declare_dram_parameter("w2", [H, H2], F32R, isOutput=False)
    w3 = nc.declare_dram_parameter("w3", [128, KBLK], F32R, isOutput=False)
    b1 = nc.declare_dram_parameter("b1", [128, HBLK], F32, isOutput=False)
    b2 = nc.declare_dram_parameter("b2", [128, KBLK], F32, isOutput=False)
    b3 = nc.declare_dram_parameter("b3", [1, 1], F32, isOutput=False)
    y = nc.declare_dram_parameter("y", [1, batch], F32, isOutput=True)

    with TileContext(nc) as tc:
        with (
            tc.tile_pool(name="wpool", bufs=1) as wpool,
            tc.tile_pool(name="xpool", bufs=1) as xpool,
            tc.tile_pool(name="hpool", bufs=1) as hpool,
            tc.tile_pool(name="ypool", bufs=4) as ypool,
            tc.tile_pool(name="pp1", bufs=3, space="PSUM") as pp1,
            tc.tile_pool(name="pp2", bufs=3, space="PSUM") as pp2,
            tc.tile_pool(name="pp3", bufs=1, space="PSUM") as pp3,
        ):
            # Weights resident in SBUF for the whole kernel. w1 arrives as 16
            # h-block slices so GEMM1 of chunk 0 can start once x(0) and the
            # first slice land (~2.5MB) instead of after the full 10MB.
            w1_sb = wpool.tile([128, DBLK, H], F32R)
            w1_r = w1.rearrange("(a p) h -> p a h", p=128)
            w3_sb = wpool.tile([128, KBLK], F32R)
            nc.sync.dma_start(out=w3_sb, in_=w3[:, :])
            b1_sb = wpool.tile([128, HBLK], F32)
            nc.sync.dma_start(out=b1_sb, in_=b1[:, :])
            b2_sb = wpool.tile([128, KBLK], F32)
            nc.sync.dma_start(out=b2_sb, in_=b2[:, :])
            b3_sb = wpool.tile([1, 1], F32)
            nc.sync.dma_start(out=b3_sb, in_=b3[:, :])
            w2_sb = wpool.tile([128, HBLK, H2], F32R)

            # Walrus allows only one semaphore wait on an fp32r Matmult, and
            # ACTIVATE waits are precious too. These 1x1 "absorber" ops consume
            # each weight/bias DMA-done semaphore on the PE/ACT clocks so the
            # first real consumer needs at most one new wait.
            scratch = wpool.tile([1, 4], F32, name="scratch")

            def absorb_pe(w_elem):
                # Standalone LDWEIGHTS: consumes the DMA-done semaphore on the
                # PE clock with no PSUM output (so no WAW serialization wait).
                # The junk weight load is overwritten by the next real
                # (self-loading fp32r) matmul.
                nc.tensor.ldweights(w_elem.bitcast(mybir.dt.bfloat16))

            def absorb_act(b_elem, i):
                nc.scalar.activation(scratch[0:1, i : i + 1], b_elem, AF.Copy)

            absorb_act(b1_sb[0:1, 0:1], 0)
            absorb_act(b2_sb[0:1, 0:1], 1)
            absorb_act(b3_sb[0:1, 0:1], 2)

            xT_r = xT.rearrange("(a p) (c n) -> p a c n", p=128, n=NB)

            for c in range(nchunk):
                x_sb = xpool.tile([128, DBLK, NB], F32R, name="x_sb")
                nc.sync.dma_start(out=x_sb, in_=xT_r[:, :, c, :])
                if c == 0:
                    for hb in range(HBLK):
                        hs = slice(hb * 128, (hb + 1) * 128)
                        nc.sync.dma_start(out=w1_sb[:, :, hs], in_=w1_r[:, :, hs])
                    # w2 is first needed by GEMM2 of chunk 0; issuing its load
                    # after x(0)/w1 lets GEMM1 start ~16MB of DMA earlier.
                    nc.sync.dma_start(
                        out=w2_sb, in_=w2.rearrange("(a p) k -> p a k", p=128)
                    )

                if c == 0:
                    absorb_pe(w1_sb[0:1, 0, 0:1])

                # GEMM1: h1T[h, b] = relu(W1.T @ xT + b1), h on partitions.
                h1_sb = hpool.tile([128, HBLK, NB], F32R, name="h1_sb")
                for hb in range(HBLK):
                    ps1 = pp1.tile([128, NB], F32, name="ps1")
                    for db in range(DBLK):
                        nc.tensor.matmul(
                            ps1,
                            w1_sb[:, db, hb * 128 : (hb + 1) * 128],
                            x_sb[:, db, :],
                            start=(db == 0),
                            stop=(db == DBLK - 1),
                        )
                    nc.scalar.activation(
                        h1_sb[:, hb, :], ps1, AF.Relu, bias=b1_sb[:, hb : hb + 1]
                    )

                if c == 0:
                    absorb_pe(w2_sb[0:1, 0, 0:1])

                # GEMM2: h2T[k, b] = relu(W2.T @ h1T + b2), k on partitions.
                h2_sb = hpool.tile([128, KBLK, NB], F32R, name="h2_sb")
                for kb in range(KBLK):
                    ps2 = pp2.tile([128, NB], F32, name="ps2")
                    for hb in range(HBLK):
                        nc.tensor.matmul(
                            ps2,
                            w2_sb[:, hb, kb * 128 : (kb + 1) * 128],
                            h1_sb[:, hb, :],
                            start=(hb == 0),
                            stop=(hb == HBLK - 1),
                        )
                    nc.scalar.activation(
                        h2_sb[:, kb, :], ps2, AF.Relu, bias=b2_sb[:, kb : kb + 1]
                    )

                if c == 0:
                    absorb_pe(w3_sb[0:1, 0:1])

                # GEMM3: yT[0, b] = sigmoid(W3.T @ h2T + b3).
                ps3 = pp3.tile([1, NB], F32, name="ps3")
                for kb in range(KBLK):
                    nc.tensor.matmul(
                        ps3,
                        w3_sb[:, kb : kb + 1],
                        h2_sb[:, kb, :],
                        start=(kb == 0),
                        stop=(kb == KBLK - 1),
                    )
                y_sb = ypool.tile([1, NB], F32, name="y_sb")
                nc.scalar.activation(y_sb, ps3, AF.Sigmoid, bias=b3_sb[0:1, 0:1])
                nc.sync.dma_start(out=y[:, c * NB : (c + 1) * NB], in_=y_sb)

    nc.finalize()
    return nc


def round_fp32r(a: np.ndarray) -> np.ndarray:
    """Round fp32 values to the FP32r grid (11-bit stored mantissa: IEEE fp32
    with the low 12 mantissa bits zero), round-to-nearest-even."""
    a = np.ascontiguousarray(a, dtype=np.float32)
    u = a.view(np.uint32)
    hi = u >> np.uint32(12)
    low = u & np.uint32(0xFFF)
    round_up = (low > np.uint32(0x800)) | (
        (low == np.uint32(0x800)) & ((hi & np.uint32(1)) == np.uint32(1))
    )
    out = ((hi + round_up.astype(np.uint32)) << np.uint32(12)).view(np.float32)
    return out


def make_in_maps(
    x: np.ndarray,
    W1: np.ndarray,
    b1: np.ndarray,
    W2: np.ndarray,
    b2: np.ndarray,
    W3: np.ndarray,
    b3: np.ndarray,
) -> list[dict[str, np.ndarray]]:
    xT = round_fp32r(np.asarray(x, dtype=np.float32).T)
    in_maps = []
    for e in range(E):
        in_maps.append(
            {
                "xT": xT,
                "w1": round_fp32r(W1[e]),
                "w2": round_fp32r(W2[e]),
                "w3": round_fp32r(W3[e].reshape(KBLK, 128).T),
                "b1": np.ascontiguousarray(b1[e].reshape(HBLK, 128).T.astype(np.float32)),
                "b2": np.ascontiguousarray(b2[e].reshape(KBLK, 128).T.astype(np.float32)),
                "b3": np.asarray(b3[e], dtype=np.float32).reshape(1, 1),
            }
        )
    return in_maps


_NC_CACHE: dict[int, bass.Bass] = {}


def run_on_hw(in_maps, batch: int = B, **kw):
    nc = _NC_CACHE.get(batch)
    if nc is None:
        nc = build_moe_nc(batch)
        _NC_CACHE[batch] = nc
    return run_bass_kernel_spmd(nc, in_maps, list(range(E)), **kw)


def kernel(x, soft_cluster_probs, W1, b1, W2, b2, W3, b3) -> np.ndarray:
    in_maps = make_in_maps(x, W1, b1, W2, b2, W3, b3)
    res = run_on_hw(in_maps, batch=x.shape[0])
    y_all = np.stack([res.results[e]["y"][0] for e in range(E)], axis=0)  # [E, B]
    combined = np.einsum(
        "eb,be->b", y_all, np.asarray(soft_cluster_probs, dtype=np.float32)
    )
    return combined.astype(np.float32).reshape(-1, 1)
